# revision 31
# baseline (speedup 1.0000x reference)
"""AWQ W4A16-style quantized linear (nn_AWQLinear) on 8 Trainium2 NeuronCores.

y[m,n] = sum_k x[m,k] * ((wq[n,k]*scales[n,g(k)] + zeros[n,g(k)]) / cs[k]) + bias[n]

Column-parallel over out_features (8 cores, N_shard = 1376/core).

All-f16 dequant formulation (keeps every DVE op in a 2x/4x perf mode and
moves the one unavoidable 1x-rate byte conversion to the idle ACT engine):

  qb16th = qb / 16                ACT copy u8->f16 (exact: 8-bit values)
  lo16th = qb16th mod 1.0         DVE tensor_scalar (= lo/16, exact)
  wd = qb16th * srep              DVE TT f16x f16, 2x mode
  wl = lo16th * srep              DVE TT f16x f16, 2x mode
  with x-side tiles  xA = 16*x'_even - x'_odd,  xB = x'_odd:
    y = sum xA^T wl + sum xB^T wd
  (identity: hi = qb16th - lo16th, even term = (16 x'_e) * (lo/16) * s)

x-side ops are DVE tensor_scalar 4x / TT 2x. Group sums use 0/1-pattern
matmuls with coefficients 1/16 (xA) and 17/16 (xB); zeros+bias fold into one
augmented matmul at the end. GPSIMD is left idle: it shares SBUF ports with
DVE and concurrent use degrades DVE ~2.5x (measured).
"""
import numpy as np

import concourse.bacc as bacc
import concourse.mybir as mybir
from concourse import tile
from concourse.bass_utils import run_bass_kernel_spmd

IN_F = 4096          # K
OUT_F = 11008        # N
M_TOK = 256          # M
NCORES = 8
NSH = OUT_F // NCORES   # 1376
NPAIR = IN_F // 256     # 16 byte-row blocks of 128 rows (each -> 2 k-tiles)
NSP = NPAIR // 2        # 8 super-pairs
CHUNKS = [(0, 512), (512, 512), (1024, NSH - 1024)]
LOOKAHEAD = 2  # super-pairs of dequant emitted ahead of their matmuls

F32, F16, U8, U16 = mybir.dt.float32, mybir.dt.float16, mybir.dt.uint8, mybir.dt.uint16


def _build_nc():
    nc = bacc.Bacc("TRN2", target_bir_lowering=False, debug=False,
                   num_devices=NCORES)

    xT_d = nc.dram_tensor("xT", [128, 32 * M_TOK], F16, kind="ExternalInput")
    csT_d = nc.dram_tensor("csT", [128, 32], F32, kind="ExternalInput")
    qwT_d = nc.dram_tensor("qwT", [IN_F // 2, NSH], U8, kind="ExternalInput")
    srep_d = nc.dram_tensor("srep", [NPAIR * 128, NSH], F16, kind="ExternalInput")
    zrT_d = nc.dram_tensor("zerosT", [32, NSH], F32, kind="ExternalInput")
    bias_d = nc.dram_tensor("bias", [1, NSH], F32, kind="ExternalInput")
    gpat_d = nc.dram_tensor("gpat", [128, 2 * NPAIR * 32], F16,
                            kind="ExternalInput")
    y_d = nc.dram_tensor("y", [M_TOK, NSH], F32, kind="ExternalOutput")

    A = mybir.AluOpType

    with tile.TileContext(nc) as tc:
        with (
            tc.tile_pool(name="const", bufs=1) as cpool,
            tc.tile_pool(name="xop", bufs=1) as xpool,
            tc.tile_pool(name="qb", bufs=3) as qbpool,
            tc.tile_pool(name="srep", bufs=3) as sreppool,
            tc.tile_pool(name="q16", bufs=2) as q16pool,
            tc.tile_pool(name="w", bufs=4) as wpool,
            tc.tile_pool(name="yout", bufs=2) as ypool,
            tc.tile_pool(name="ps", bufs=1, space="PSUM") as pspool,
        ):
            # ---- hot-path constants only (tail constants loaded later) ----
            csT = cpool.tile([128, 32], F32)
            nc.scalar.dma_start(csT[:], csT_d[:])
            rcs = cpool.tile([128, 32], F32)
            nc.vector.reciprocal(rcs[:], csT[:])

            gpat = cpool.tile([128, 2 * NPAIR * 32], F16)

            # x tiles allocated now; their DMAs are issued after the first
            # dequant block so qb/srep for sp0 win the sync-queue head.
            w4 = 4 * M_TOK
            xraw = [xpool.tile([128, w4], F16, tag=f"xraw_{c}",
                               name=f"xraw__{c}") for c in range(8)]

            def xslice(t):
                return xraw[t // 4][:, (t % 4) * M_TOK:(t % 4 + 1) * M_TOK]

            def emit_x_dmas():
                nc.sync.dma_start(xraw[0][:], xT_d[:, 0:w4])
                nc.sync.dma_start(gpat[:], gpat_d[:])
                for c in range(1, 8):
                    nc.sync.dma_start(xraw[c][:], xT_d[:, c * w4:(c + 1) * w4])

            # ---- psum accumulators ----
            y_ps = [[pspool.tile([128, w], F32, tag=f"yps_{m}_{ci}",
                                 name=f"yps_{m}_{ci}")
                     for ci, (_, w) in enumerate(CHUNKS)] for m in range(2)]
            S_ps = pspool.tile([32, M_TOK], F32, tag="S_ps")

            # ---- software-pipelined main loop ----
            state = {}   # sp -> (wl, wd, {b: (xA, xB)})
            ACT_LO = {2, 4, 6}   # SPs whose lo-nibble f16 convert runs on ACT

            sp_inputs = {}

            def load_sp_inputs(sp):
                if sp in sp_inputs:
                    return sp_inputs.pop(sp)
                qbcat = qbpool.tile([128, 2 * NSH], U8, tag="qb",
                                    name=f"qb_{sp}")
                nc.sync.dma_start(
                    qbcat[:].rearrange("p (j n) -> p j n", j=2),
                    qwT_d[sp * 256:(sp + 1) * 256, :]
                    .rearrange("(j p) n -> p j n", p=128))
                srepc = sreppool.tile([128, 2 * NSH], F16, tag="srep",
                                      name=f"srep_{sp}")
                nc.sync.dma_start(
                    srepc[:].rearrange("p (j n) -> p j n", j=2),
                    srep_d[sp * 256:(sp + 1) * 256, :]
                    .rearrange("(j p) n -> p j n", p=128))
                return qbcat, srepc

            sp0_tiles = []

            def emit_sp0_dmas():
                for j in range(2):
                    r0 = j * 128
                    qb = qbpool.tile([128, NSH], U8, tag="qb0",
                                     name=f"qb0_{j}")
                    nc.sync.dma_start(qb[:], qwT_d[r0:r0 + 128, :])
                    sr = sreppool.tile([128, NSH], F16, tag="srep0",
                                       name=f"srep0_{j}")
                    nc.sync.dma_start(sr[:], srep_d[r0:r0 + 128, :])
                    sp0_tiles.append((qb, sr))

            def emit_dequant_sp0():
                # fine-grained first super-pair: per-pair DMAs/extracts/TTs so
                # the first matmuls unblock as early as possible
                wsl, xab = {}, {}
                for j in range(2):
                    b = j
                    qb, sr = sp0_tiles[j]
                    lo8 = q16pool.tile([128, NSH], U8, tag="lo8_0",
                                       name=f"lo8_0_{j}")
                    nc.vector.tensor_scalar(lo8[:].bitcast(U16),
                                            in0=qb[:].bitcast(U16),
                                            scalar1=0x0F0F, scalar2=None,
                                            op0=A.bitwise_and)
                    wl = wpool.tile([128, NSH], F16, tag="wl0",
                                    name=f"wl0_{j}")
                    nc.vector.tensor_tensor(wl[:], lo8[:], sr[:], A.mult)
                    q16 = q16pool.tile([128, NSH], F16, tag="q16_0",
                                       name=f"q16_0_{j}")
                    nc.scalar.mul(q16[:], qb[:], 0.0625)
                    wd = wpool.tile([128, NSH], F16, tag="wd0",
                                    name=f"wd0_{j}")
                    nc.vector.tensor_tensor(wd[:], q16[:], sr[:], A.mult)
                    wsl[(j, "l")] = wl[:]
                    wsl[(j, "d")] = wd[:]
                    te, to = 2 * b, 2 * b + 1
                    xB = xpool.tile([128, M_TOK], F16, tag="xB",
                                    bufs=6, name=f"xB_{b}")
                    nc.vector.tensor_scalar(xB[:], in0=xslice(to),
                                            scalar1=rcs[:, to:to + 1],
                                            scalar2=None, op0=A.mult)
                    xC = xpool.tile([128, M_TOK], F16, tag="xC",
                                    bufs=3, name=f"xC_{b}")
                    nc.vector.tensor_scalar(xC[:], in0=xB[:], scalar1=0.0625,
                                            scalar2=None, op0=A.mult)
                    xA = xpool.tile([128, M_TOK], F16, tag="xA",
                                    bufs=6, name=f"xA_{b}")
                    nc.vector.scalar_tensor_tensor(
                        xA[:], in0=xslice(te), scalar=rcs[:, te:te + 1],
                        in1=xC[:], op0=A.mult, op1=A.subtract)
                    xab[b] = (xA, xB)
                state[0] = (wsl, xab)

            def emit_dequant(sp):
                if sp == 0:
                    emit_dequant_sp0()
                    return
                qbcat, srepc = load_sp_inputs(sp)

                # lo nibbles: u16 fused bitwise extract (2x mode)
                lo8 = q16pool.tile([128, 2 * NSH], U8, tag="lo8",
                                   name=f"lo8_{sp}")
                nc.vector.tensor_scalar(lo8[:].bitcast(U16),
                                        in0=qbcat[:].bitcast(U16),
                                        scalar1=0x0F0F, scalar2=None,
                                        op0=A.bitwise_and)
                wl = wpool.tile([128, 2 * NSH], F16, tag="wl",
                                name=f"wl_{sp}")
                if sp in ACT_LO:
                    lo16 = q16pool.tile([128, 2 * NSH], F16, tag="lo16",
                                        name=f"lo16_{sp}")
                    nc.scalar.copy(lo16[:], lo8[:])
                    nc.vector.tensor_tensor(wl[:], lo16[:], srepc[:], A.mult)
                else:
                    nc.vector.tensor_tensor(wl[:], lo8[:], srepc[:], A.mult)

                # byte -> f16 conversion on ACT (scale 1/16: exact values)
                q16 = q16pool.tile([128, 2 * NSH], F16, tag="q16",
                                   name=f"q16_{sp}")
                nc.scalar.mul(q16[:], qbcat[:], 0.0625)
                wd = wpool.tile([128, 2 * NSH], F16, tag="wd",
                                name=f"wd_{sp}")
                nc.vector.tensor_tensor(wd[:], q16[:], srepc[:], A.mult)

                xab = {}
                for j in range(2):
                    b = 2 * sp + j
                    te, to = 2 * b, 2 * b + 1
                    xB = xpool.tile([128, M_TOK], F16, tag="xB",
                                    bufs=6, name=f"xB_{b}")
                    nc.vector.tensor_scalar(xB[:], in0=xslice(to),
                                            scalar1=rcs[:, to:to + 1],
                                            scalar2=None, op0=A.mult)
                    xC = xpool.tile([128, M_TOK], F16, tag="xC",
                                    bufs=3, name=f"xC_{b}")
                    nc.vector.tensor_scalar(xC[:], in0=xB[:], scalar1=0.0625,
                                            scalar2=None, op0=A.mult)
                    xA = xpool.tile([128, M_TOK], F16, tag="xA",
                                    bufs=6, name=f"xA_{b}")
                    nc.vector.scalar_tensor_tensor(
                        xA[:], in0=xslice(te), scalar=rcs[:, te:te + 1],
                        in1=xC[:], op0=A.mult, op1=A.subtract)
                    xab[b] = (xA, xB)
                wsl = {}
                for j in range(2):
                    wsl[(j, "l")] = wl[:, j * NSH:(j + 1) * NSH]
                    wsl[(j, "d")] = wd[:, j * NSH:(j + 1) * NSH]
                state[sp] = (wsl, xab)

            def emit_mms(sp):
                last = (sp == NSP - 1)
                wsl, xab = state.pop(sp)
                for j in range(2):
                    b = 2 * sp + j
                    xA, xB = xab[b]
                    nc.tensor.matmul(S_ps[:],
                                     gpat[:, (2 * b) * 32:(2 * b + 1) * 32],
                                     xA[:], start=(b == 0), stop=False)
                    nc.tensor.matmul(S_ps[:],
                                     gpat[:, (2 * b + 1) * 32:(2 * b + 2) * 32],
                                     xB[:], start=False,
                                     stop=(b == NPAIR - 1))
                    if last:
                        continue
                    for kind, xt in (("l", xA), ("d", xB)):
                        w = wsl[(j, kind)]
                        for m in range(2):
                            for ci, (c0, cw) in enumerate(CHUNKS):
                                nc.tensor.matmul(
                                    y_ps[m][ci][:],
                                    xt[:, m * 128:(m + 1) * 128],
                                    w[:, c0:c0 + cw],
                                    start=(b == 0 and kind == "l"),
                                    stop=False,
                                )
                if not last:
                    return
                # last SP: S16 copy overlaps main MMs; per-chunk zeros-MM +
                # drain + store overlap the remaining chunks' matmuls
                S16 = cpool.tile([33, M_TOK], F16)
                nc.scalar.copy(S16[:32, :], S_ps[:])
                nc.vector.memset(S16[32:33, :], 1.0)
                for ci, (c0, cw) in enumerate(CHUNKS):
                    for j in range(2):
                        b = 2 * sp + j
                        xA, xB = xab[b]
                        for kind, xt in (("l", xA), ("d", xB)):
                            w = wsl[(j, kind)]
                            for m in range(2):
                                nc.tensor.matmul(
                                    y_ps[m][ci][:],
                                    xt[:, m * 128:(m + 1) * 128],
                                    w[:, c0:c0 + cw],
                                    start=False, stop=False)
                    for m in range(2):
                        nc.tensor.matmul(y_ps[m][ci][:],
                                         S16[:, m * 128:(m + 1) * 128],
                                         zT16[:, c0:c0 + cw],
                                         start=False, stop=True)
                        ysb = ypool.tile([128, cw], F32, tag=f"ysb_{ci}",
                                         name=f"ysb_{m}_{ci}")
                        nc.scalar.copy(ysb[:], y_ps[m][ci][:])
                        nc.sync.dma_start(
                            y_d[m * 128:(m + 1) * 128, c0:c0 + cw], ysb[:])

            emit_sp0_dmas()
            emit_x_dmas()
            for sp in range(NSP + LOOKAHEAD):
                if sp < NSP:
                    emit_dequant(sp)
                if sp == 2:
                    # tail-only constants: emitted after the hot path kickoff
                    zrT32 = cpool.tile([32, NSH], F32)
                    nc.sync.dma_start(zrT32[:], zrT_d[:])
                    zT16 = cpool.tile([33, NSH], F16)
                    nc.scalar.copy(zT16[:32, :], zrT32[:])
                    b32 = cpool.tile([1, NSH], F32)
                    nc.sync.dma_start(b32[:], bias_d[:])
                    nc.scalar.copy(zT16[32:33, :], b32[:])
                if sp >= LOOKAHEAD:
                    emit_mms(sp - LOOKAHEAD)

    nc.compile()
    return nc


def _host_prep(x, qweight, scales, zeros, channel_scales, bias):
    x2 = np.asarray(x, dtype=np.float32).reshape(M_TOK, IN_F)
    qw = np.asarray(qweight)
    if qw.dtype != np.uint8:
        qw = qw.astype(np.uint8)
    qwT = np.ascontiguousarray(qw.T)                      # [K/2, N]

    q = np.arange(128)
    perm = np.empty(IN_F, np.int64)
    for b in range(NPAIR):
        perm[(2 * b) * 128 + q] = 256 * b + 2 * q
        perm[(2 * b + 1) * 128 + q] = 256 * b + 2 * q + 1

    xT_perm = x2.T[perm]                                  # [K, M]
    xT_b = np.ascontiguousarray(
        xT_perm.reshape(32, 128, M_TOK).transpose(1, 0, 2)
        .reshape(128, 32 * M_TOK)).astype(np.float16)
    cs_perm = np.asarray(channel_scales, np.float32)[perm]
    csT = np.ascontiguousarray(cs_perm.reshape(32, 128).T)  # [128, 32]

    scalesT = np.asarray(scales, np.float32).T            # [32, N]
    srep = np.empty((NPAIR * 128, OUT_F), np.float16)
    for b in range(NPAIR):
        srep[b * 128:b * 128 + 64] = scalesT[2 * b].astype(np.float16)
        srep[b * 128 + 64:(b + 1) * 128] = scalesT[2 * b + 1].astype(np.float16)

    zerosT = np.ascontiguousarray(np.asarray(zeros, np.float32).T)
    bias_f = np.asarray(bias, np.float32)

    # per-pair patterns: block 2b for xA (coeff 1), block 2b+1 for xB (17/16)
    gpat = np.zeros((128, 2 * NPAIR * 32), np.float16)
    for b in range(NPAIR):
        for blk, val in ((2 * b, 1.0), (2 * b + 1, 1.0625)):
            gpat[0:64, blk * 32 + 2 * b] = val
            gpat[64:128, blk * 32 + 2 * b + 1] = val

    return xT_b, csT, qwT, srep, zerosT, bias_f, gpat


def make_in_maps(x, qweight, scales, zeros, channel_scales, bias):
    xT_b, csT, qwT, srep, zerosT, bias_f, gpat = _host_prep(
        x, qweight, scales, zeros, channel_scales, bias)
    in_maps = []
    for c in range(NCORES):
        sl = slice(c * NSH, (c + 1) * NSH)
        in_maps.append({
            "xT": xT_b,
            "csT": csT,
            "qwT": np.ascontiguousarray(qwT[:, sl]),
            "srep": np.ascontiguousarray(srep[:, sl]),
            "zerosT": np.ascontiguousarray(zerosT[:, sl]),
            "bias": np.ascontiguousarray(bias_f[sl]).reshape(1, NSH),
            "gpat": gpat,
        })
    return in_maps


_NC_CACHE = {}


def get_nc():
    if "nc" not in _NC_CACHE:
        _NC_CACHE["nc"] = _build_nc()
    return _NC_CACHE["nc"]


def kernel(x, qweight, scales, zeros, channel_scales, bias):
    in_maps = make_in_maps(x, qweight, scales, zeros, channel_scales, bias)
    nc = get_nc()
    res = run_bass_kernel_spmd(nc, in_maps, core_ids=list(range(NCORES)))
    y = np.concatenate([res.results[c]["y"] for c in range(NCORES)], axis=1)
    return y.reshape(1, M_TOK, OUT_F).astype(np.float32)
